# revision 1
# baseline (speedup 1.0000x reference)
"""Trainium2 Bass kernel: per-superpixel mean of CNN features + linear head.

reference computes:
    sums[s, f]  = segment_sum(features, superpixel)      # 1024 segments
    out[s, c]   = (sums[s] / max(count_s, 1)) @ w_node.T # [1024, 21]

Key algebraic restructure: project each pixel's 256-dim feature to the
22-dim augmented class space FIRST (21 classes + a ones-column that
yields the segment counts), then segment-sum the projections:
    out[s, c] = segsum(feats @ w_aug.T)[s, c] / segsum(ones)[s]
This turns the segment reduction into a [pix,22].T @ onehot[pix,1024]
matmul per 128-pixel tile, accumulated in PSUM across all tiles.

Sharding: the 512*512 = 262144 pixels are split evenly across 8 cores
(segment-sum is permutation-invariant over pixels). Each core emits a
[rows, 1024] partial (class sums + counts); the host adds the partials,
divides by counts and transposes.
"""

import numpy as np

import concourse.mybir as mybir
import concourse.tile as tile
from concourse import bacc
from concourse.bass_utils import run_bass_kernel_spmd

N_CORES = 8
P = 128
F = 256                      # feature dim
NUM_SP = 1024                # superpixel labels
C = 21                       # classes
CP = 22                      # classes padded even (fp32r needs even moving dim)
NPIX = 512 * 512
PIX_PER_CORE = NPIX // N_CORES       # 32768
import os as _os

CHUNK_PIX = int(_os.environ.get("KERNEL_CHUNK_PIX", "2048"))  # pixels per DMA chunk
N_CHUNKS = PIX_PER_CORE // CHUNK_PIX
TILES_PER_CHUNK = CHUNK_PIX // P
N_TILES = PIX_PER_CORE // P           # 256
FREE_PER_CHUNK = CHUNK_PIX * F // P

F32 = mybir.dt.float32
F32R = mybir.dt.float32r  # fp32 layout, full-rate PE path
BF16 = mybir.dt.bfloat16
I16 = mybir.dt.int16

# segment-sum matmuls rotate over PE column-tiling groups so consecutive
# tiles' matmuls overlap in disjoint 32-column strips of the array
N_GROUPS = 4


def _build_nc():
    import os

    use_lo = bool(int(os.environ.get("KERNEL_LO", "1")))
    merge = bool(int(os.environ.get("KERNEL_MERGE", "0")))
    bhl = bool(int(os.environ.get("KERNEL_BHL", "0")))
    swp = bool(int(os.environ.get("KERNEL_SWP", "0")))
    split_evac = bool(int(os.environ.get("KERNEL_SPLIT_EVAC", "0")))
    split_first = bool(int(os.environ.get("KERNEL_SPLIT_FIRST", "1")))
    work_bufs = int(os.environ.get("KERNEL_WORK_BUFS", "6"))
    psum_bufs = int(os.environ.get("KERNEL_PSUM_BUFS", "2"))
    chunk_bufs = int(os.environ.get("KERNEL_CHUNK_BUFS", "3"))
    nc = bacc.Bacc("TRN2", target_bir_lowering=False)

    feats = nc.dram_tensor(
        "feats", [N_CHUNKS, P, FREE_PER_CHUNK], F32 if bhl else F32R,
        kind="ExternalInput",
    )
    labels = nc.dram_tensor("labels", [P, N_TILES], F32, kind="ExternalInput")
    iota = nc.dram_tensor("iota", [P, NUM_SP], I16, kind="ExternalInput")
    w_aug = nc.dram_tensor("w_aug", [2 * P, CP], F32R, kind="ExternalInput")
    ident_d = nc.dram_tensor("ident", [P, P], BF16 if bhl else F32R, kind="ExternalInput")
    out = nc.dram_tensor("out", [P, NUM_SP], F32, kind="ExternalOutput")

    with tile.TileContext(nc) as tc:
        with (
            tc.tile_pool(name="const", bufs=1) as const_pool,
            tc.tile_pool(name="chunk", bufs=chunk_bufs) as chunk_pool,
            tc.tile_pool(name="work", bufs=work_bufs) as work_pool,
            tc.tile_pool(name="psum", bufs=psum_bufs, space="PSUM") as psum_pool,
            tc.tile_pool(name="accp", bufs=1, space="PSUM") as acc_pool,
        ):
            ident = const_pool.tile([P, P], BF16 if bhl else F32R)
            nc.sync.dma_start(out=ident[:], in_=ident_d[:])
            iota_sb = const_pool.tile([P, NUM_SP], I16)
            nc.sync.dma_start(out=iota_sb[:], in_=iota[:])
            labels_sb = const_pool.tile([P, N_TILES], F32)
            nc.sync.dma_start(out=labels_sb[:], in_=labels[:])
            w_sb = const_pool.tile([P, 2 * CP], F32R)
            nc.sync.dma_start(out=w_sb[:, 0:CP], in_=w_aug[0:P, :])
            nc.sync.dma_start(out=w_sb[:, CP : 2 * CP], in_=w_aug[P : 2 * P, :])

            # persistent accumulator: group j accumulates into rows
            # [32j, 32j+CAUG) across its subset of pixel tiles
            acc = acc_pool.tile([P, NUM_SP], F32)

            def emit_segsum(pq_sb, lo_sb, onehot, tg):
                # acc[row + c, s] += part[pix, c] * onehot[pix, s]
                if use_lo:
                    parts = ((pq_sb, 2 * (tg % 2)), (lo_sb, 2 * (tg % 2) + 1))
                    first = tg < 2
                    last = tg >= N_TILES - 2
                else:
                    parts = ((pq_sb, tg % N_GROUPS),)
                    first = tg < N_GROUPS
                    last = tg >= N_TILES - N_GROUPS
                for part, g in parts:
                    row = 32 * g
                    for half in range(2):
                        nc.tensor.matmul(
                            out=acc[row : row + CP, 512 * half : 512 * (half + 1)],
                            lhsT=part[:],
                            rhs=onehot[:, 512 * half : 512 * (half + 1)],
                            start=first,
                            stop=last,
                            tile_position=(0, row),
                            skip_group_check=True,
                        )

            pending = None

            for c in range(N_CHUNKS):
                feats_sb = chunk_pool.tile(
                    [P, FREE_PER_CHUNK], F32 if bhl else F32R, tag="feats"
                )
                if c == 0 and split_first:
                    # first chunk in four sub-DMAs so tile 0's compute can
                    # start after the first quarter lands (shorter ramp)
                    q = FREE_PER_CHUNK // 4
                    for k in range(4):
                        nc.sync.dma_start(
                            out=feats_sb[:, k * q : (k + 1) * q],
                            in_=feats[c][:, k * q : (k + 1) * q],
                        )
                else:
                    nc.sync.dma_start(out=feats_sb[:], in_=feats[c])
                if bhl:
                    # chunk-level bf16 hi/lo split of the features; hi+lo
                    # reconstructs fp32 exactly when accumulated in PSUM
                    fhi_sb = chunk_pool.tile([P, FREE_PER_CHUNK], BF16, tag="fhi")
                    nc.scalar.activation(
                        out=fhi_sb[:],
                        in_=feats_sb[:],
                        func=mybir.ActivationFunctionType.Copy,
                    )
                    flo_sb = chunk_pool.tile([P, FREE_PER_CHUNK], BF16, tag="flo")
                    nc.gpsimd.tensor_tensor(
                        out=flo_sb[:],
                        in0=feats_sb[:],
                        in1=fhi_sb[:],
                        op=mybir.AluOpType.subtract,
                    )
                for t in range(TILES_PER_CHUNK):
                    tg = c * TILES_PER_CHUNK + t
                    fcol = t * F

                    # transpose the [128 pix, 256 f] tile -> [256 f, 128 pix]
                    if bhl:
                        # plain bf16 matmuls (keep the PE HAM-warm, unlike
                        # transpose-mode): ft = fhi.T @ I + flo.T @ I
                        ft_ps = psum_pool.tile([P, F], F32, tag="ftps")
                        for b in range(2):
                            lo_c = fcol + P * b
                            nc.tensor.matmul(
                                out=ft_ps[:, P * b : P * (b + 1)],
                                lhsT=fhi_sb[:, lo_c : lo_c + P],
                                rhs=ident[:],
                                start=True,
                                stop=False,
                                skip_group_check=True,
                            )
                            nc.tensor.matmul(
                                out=ft_ps[:, P * b : P * (b + 1)],
                                lhsT=flo_sb[:, lo_c : lo_c + P],
                                rhs=ident[:],
                                start=False,
                                stop=True,
                                skip_group_check=True,
                            )
                    else:
                        ft_ps = psum_pool.tile([P, F], F32R, tag="ftps")
                        nc.tensor.transpose(
                            out=ft_ps[:, 0:P],
                            in_=feats_sb[:, fcol : fcol + P],
                            identity=ident[:],
                        )
                        nc.tensor.transpose(
                            out=ft_ps[:, P:F],
                            in_=feats_sb[:, fcol + P : fcol + F],
                            identity=ident[:],
                        )
                    ft_sb = work_pool.tile([P, F], F32R, tag="ftsb")
                    if split_evac:
                        # two half evacs: proj MM on block 0 can start
                        # while block 1 is still copying out of PSUM
                        nc.scalar.activation(
                            out=ft_sb[:, 0:P],
                            in_=ft_ps[:, 0:P],
                            func=mybir.ActivationFunctionType.Copy,
                        )
                        nc.scalar.activation(
                            out=ft_sb[:, P:F],
                            in_=ft_ps[:, P:F],
                            func=mybir.ActivationFunctionType.Copy,
                        )
                    else:
                        nc.scalar.activation(
                            out=ft_sb[:],
                            in_=ft_ps[:],
                            func=mybir.ActivationFunctionType.Copy,
                        )

                    # proj[pix, 22] = feats @ w_aug.T  (contract over features)
                    proj_ps = psum_pool.tile([P, CP], F32, tag="projps")
                    nc.tensor.matmul(
                        out=proj_ps[:],
                        lhsT=ft_sb[:, 0:P],
                        rhs=w_sb[:, 0:CP],
                        start=True,
                        stop=False,
                    )
                    nc.tensor.matmul(
                        out=proj_ps[:],
                        lhsT=ft_sb[:, P:F],
                        rhs=w_sb[:, CP : 2 * CP],
                        start=False,
                        stop=True,
                    )
                    # bf16 proj for the segment-sum matmul; PSUM accumulates fp32
                    if merge:
                        pq_sb = work_pool.tile([P, 2 * CP], BF16, tag="pqsb")
                        hi_ap, lo_ap = pq_sb[:, 0:CP], pq_sb[:, CP : 2 * CP]
                    else:
                        pq_sb = work_pool.tile([P, CP], BF16, tag="pqsb")
                        hi_ap = pq_sb[:]
                        if use_lo:
                            lo_sb = work_pool.tile([P, CP], BF16, tag="losb")
                            lo_ap = lo_sb[:]
                    nc.scalar.activation(
                        out=hi_ap,
                        in_=proj_ps[:],
                        func=mybir.ActivationFunctionType.Copy,
                    )
                    if use_lo or merge:
                        nc.vector.tensor_tensor(
                            out=lo_ap,
                            in0=proj_ps[:],
                            in1=hi_ap,
                            op=mybir.AluOpType.subtract,
                        )

                    # onehot[p, s] = (iota[p, s] == label[p]); int16 input
                    # enables the DVE 4x mode, bf16 output feeds the PE
                    onehot = work_pool.tile([P, NUM_SP], BF16, tag="onehot")
                    nc.vector.tensor_scalar(
                        onehot[:],
                        iota_sb[:],
                        labels_sb[:, tg : tg + 1],
                        None,
                        mybir.AluOpType.is_equal,
                    )

                    # acc[row + c, s] += pq[pix, c] * onehot[pix, s]
                    if merge:
                        # hi|lo side by side: one [128, 44] stationary per tile,
                        # groups alternate partitions {0, 64}
                        row = 64 * (tg % 2)
                        first = tg < 2
                        last = tg >= N_TILES - 2
                        for half in range(2):
                            nc.tensor.matmul(
                                out=acc[row : row + 2 * CP, 512 * half : 512 * (half + 1)],
                                lhsT=pq_sb[:],
                                rhs=onehot[:, 512 * half : 512 * (half + 1)],
                                start=first,
                                stop=last,
                                tile_position=(0, row),
                                skip_group_check=True,
                            )
                    elif swp:
                        # software pipeline: emit tile t-1's segment-sum
                        # AFTER tile t's transposes/proj so the strict-FIFO
                        # PE queue has independent work if operands lag
                        if pending is not None:
                            emit_segsum(*pending)
                        pending = (pq_sb, lo_sb if use_lo else None, onehot, tg)
                    else:
                        emit_segsum(pq_sb, lo_sb if use_lo else None, onehot, tg)

            if pending is not None:
                emit_segsum(*pending)
            out_sb = chunk_pool.tile([P, NUM_SP], F32, tag="outsb")
            nc.scalar.activation(
                out=out_sb[:], in_=acc[:], func=mybir.ActivationFunctionType.Copy
            )
            nc.sync.dma_start(out=out[:], in_=out_sb[:])

    nc.compile()
    return nc


def _install_ntff_hook():
    """Register the axon NTFF profiling hook when the image's antenv
    lacks axon_hooks (mirrors trn_agent_boot._ntff_profile_via_ctypes)."""
    import contextlib
    import ctypes
    import sys
    import types

    if "antenv.axon_hooks" in sys.modules:
        return
    lib = ctypes.CDLL("/opt/axon/libaxon_pjrt.so")
    if not hasattr(lib, "axon_start_nrt_profile"):
        return
    lib.axon_start_nrt_profile.argtypes = [
        ctypes.POINTER(ctypes.c_int64),
        ctypes.c_size_t,
    ]
    lib.axon_start_nrt_profile.restype = ctypes.c_int64
    lib.axon_stop_nrt_profile.argtypes = [ctypes.c_char_p]
    lib.axon_stop_nrt_profile.restype = ctypes.c_int64

    @contextlib.contextmanager
    def _hook(output_dir, device_ids):
        import jax

        jax.devices()
        if device_ids:
            ids = (ctypes.c_int64 * len(device_ids))(*device_ids)
            rc = lib.axon_start_nrt_profile(ids, len(device_ids))
        else:
            rc = lib.axon_start_nrt_profile(None, 0)
        if rc != 0:
            raise RuntimeError(f"axon_start_nrt_profile rc={rc}")
        try:
            yield
        finally:
            n = lib.axon_stop_nrt_profile(str(output_dir).encode())
            print(f"profile: {n} file(s) written to {output_dir}", file=sys.stderr)

    mod = types.ModuleType("antenv.axon_hooks")
    mod.get_axon_ntff_profile_hook = lambda: _hook
    mod.set_axon_ntff_profile_hook = lambda h: None
    sys.modules["antenv.axon_hooks"] = mod


_NC_CACHE = None


def _get_nc():
    global _NC_CACHE
    if _NC_CACHE is None:
        _NC_CACHE = _build_nc()
    return _NC_CACHE


def kernel(features, superpixel, w_node):
    features = np.ascontiguousarray(np.asarray(features, dtype=np.float32))
    superpixel = np.asarray(superpixel)
    w_node = np.asarray(w_node, dtype=np.float32)

    feats_flat = features.reshape(NPIX, F)
    sp_flat = superpixel.reshape(NPIX)

    # w_aug[f, c] layout: two stacked [128, 22] blocks of
    # [w_node.T | ones] so rhs block b is w_aug[128b:128b+128, :]
    w_aug = np.zeros((F, CP), dtype=np.float32)
    w_aug[:, :C] = w_node.T
    if bool(int(_os.environ.get("KERNEL_BHL", "0"))):
        import ml_dtypes

        ident = np.eye(P, dtype=ml_dtypes.bfloat16)
    else:
        ident = np.eye(P, dtype=np.float32)
    iota = np.broadcast_to(
        np.arange(NUM_SP, dtype=np.int16)[None, :], (P, NUM_SP)
    ).copy()

    in_maps = []
    for core in range(N_CORES):
        lo = core * PIX_PER_CORE
        fc = feats_flat[lo : lo + PIX_PER_CORE]
        spc = sp_flat[lo : lo + PIX_PER_CORE]
        # pixel index within core = 2048*chunk + 16*partition + tile_in_chunk
        lab = (
            spc.reshape(N_CHUNKS, P, TILES_PER_CHUNK)
            .transpose(1, 0, 2)
            .reshape(P, N_TILES)
            .astype(np.float32)
        )
        in_maps.append(
            {
                "feats": fc.reshape(N_CHUNKS, P, FREE_PER_CHUNK),
                "labels": np.ascontiguousarray(lab),
                "iota": iota,
                "w_aug": w_aug,
                "ident": ident,
            }
        )

    import os

    trace = bool(int(os.environ.get("KERNEL_TRACE", "0")))
    repeat = int(os.environ.get("KERNEL_REPEAT", "1"))
    kwargs = {}
    if trace:
        _install_ntff_hook()
        import concourse.bass_utils as _bu

        _bu.upload_artifacts = lambda tmpdir: tmpdir
    base_dir = os.environ.get("KERNEL_TRACE_DIR") or None
    for rep in range(repeat):
        if trace and base_dir:
            kwargs["tmpdir"] = os.path.join(base_dir, f"rep{rep}")
            os.makedirs(kwargs["tmpdir"], exist_ok=True)
        res = run_bass_kernel_spmd(
            _get_nc(), in_maps, core_ids=list(range(N_CORES)), trace=trace, **kwargs
        )
        if trace:
            print(f"HW exec time: {res.exec_time_ns} ns")
            print(f"profile_json: {res.profile_json}")

    total = np.zeros((C, NUM_SP), dtype=np.float64)
    merged = bool(int(os.environ.get("KERNEL_MERGE", "0")))
    bases = (0, CP, 64, 64 + CP) if merged else (0, 32, 64, 96)
    for r in res.results:
        o = np.asarray(r["out"], dtype=np.float64)
        for b in bases:
            total += o[b : b + C]
    counts = np.bincount(sp_flat.astype(np.int64), minlength=NUM_SP).astype(np.float64)
    node_potentials = total / np.clip(counts, 1.0, None)
    return np.ascontiguousarray(node_potentials.T).astype(np.float32)



# revision 8
# speedup vs baseline: 3.1144x; 3.1144x over previous
"""Trainium2 Bass kernel: per-superpixel mean of CNN features + linear head.

reference computes:
    sums[s, f]  = segment_sum(features, superpixel)      # 1024 segments
    out[s, c]   = (sums[s] / max(count_s, 1)) @ w_node.T # [1024, 21]

Algebraic restructure: project each pixel's 256-dim feature to the 22-dim
padded class space FIRST, then segment-sum the projections:
    out[s, c] = segsum(feats @ w_aug.T)[s, c] / count_s
The segment reduction is a [pix,22].T @ onehot[pix,1024] matmul per
128-pixel tile, accumulated in PSUM across tiles.

v2 layout choices (vs v1):
  * features are transposed to [256 f, pix] bf16 on the host, so the
    projection reads fT blocks directly as the PE stationary operand —
    no per-tile PE transposes, fast (FWL) weight loads, and half the
    HBM traffic of fp32.
  * superpixel labels and the iota row are encoded as distinct bf16 BIT
    PATTERNS (0x4000+v) so the onehot is_equal compare runs all-bf16
    (DVE 4x mode eligible) yet stays exact.
  * each tile's two 512-wide segment-sum matmuls go to DIFFERENT PE
    column groups (tile parity rotates over 4 groups), so their moving
    streams overlap via separate XBUSes.

Sharding: 512*512 pixels split evenly across 8 cores (segment-sum is
permutation-invariant). Each core emits a [128, 512] partial holding 4
groups x 22 class rows; the host adds the partials, divides by counts
(np.bincount) and transposes.
"""

import os as _os

import numpy as np
import ml_dtypes

import concourse.mybir as mybir
import concourse.tile as tile
from concourse import bacc
from concourse.bass_utils import run_bass_kernel_spmd

N_CORES = 8
P = 128
F = 256                      # feature dim
NUM_SP = 1024                # superpixel labels
C = 21                       # classes
CP = 22                      # classes padded even
NPIX = 512 * 512
PIX_PER_CORE = NPIX // N_CORES       # 32768
N_TILES = PIX_PER_CORE // P          # 256

CHUNK_PIX = int(_os.environ.get("KERNEL_CHUNK_PIX", "2048"))
N_CHUNKS = PIX_PER_CORE // CHUNK_PIX
TILES_PER_CHUNK = CHUNK_PIX // P

F32 = mybir.dt.float32
BF16 = mybir.dt.bfloat16


def _build_nc():
    work_bufs = int(_os.environ.get("KERNEL_WORK_BUFS", "6"))
    psum_bufs = int(_os.environ.get("KERNEL_PSUM_BUFS", "3"))
    chunk_bufs = int(_os.environ.get("KERNEL_CHUNK_BUFS", "3"))
    split_first = bool(int(_os.environ.get("KERNEL_SPLIT_FIRST", "1")))

    nc = bacc.Bacc("TRN2", target_bir_lowering=False)

    feats = nc.dram_tensor(
        "feats", [N_CHUNKS, 2, P, CHUNK_PIX], BF16, kind="ExternalInput"
    )
    labels = nc.dram_tensor("labels", [P, N_TILES], F32, kind="ExternalInput")
    iota = nc.dram_tensor("iota", [P, NUM_SP], BF16, kind="ExternalInput")
    w_aug = nc.dram_tensor("w_aug", [P, 2, CP], BF16, kind="ExternalInput")
    out = nc.dram_tensor("out", [P, 512], F32, kind="ExternalOutput")

    with tile.TileContext(nc) as tc:
        with (
            tc.tile_pool(name="const", bufs=1) as const_pool,
            tc.tile_pool(name="chunk", bufs=chunk_bufs) as chunk_pool,
            tc.tile_pool(name="work", bufs=work_bufs) as work_pool,
            tc.tile_pool(name="psum", bufs=psum_bufs, space="PSUM") as psum_pool,
            tc.tile_pool(name="accp", bufs=1, space="PSUM") as acc_pool,
        ):
            iota_sb = const_pool.tile([P, NUM_SP], BF16)
            nc.sync.dma_start(out=iota_sb[:], in_=iota[:])
            labels_sb = const_pool.tile([P, N_TILES], F32)
            nc.sync.dma_start(out=labels_sb[:], in_=labels[:])
            w_sb = const_pool.tile([P, 2, CP], BF16)
            nc.sync.dma_start(out=w_sb[:], in_=w_aug[:])

            # persistent accumulator: group g = 2*(tg%2)+half accumulates
            # class sums for sp half `half` into partitions [32g, 32g+CP)
            acc = acc_pool.tile([P, 512], F32)

            for c in range(N_CHUNKS):
                feats_sb = chunk_pool.tile([P, 2, CHUNK_PIX], BF16, tag="feats")
                for h in range(2):
                    if c == 0 and split_first:
                        q = CHUNK_PIX // 2
                        for k in range(2):
                            nc.sync.dma_start(
                                out=feats_sb[:, h, k * q : (k + 1) * q],
                                in_=feats[c, h][:, k * q : (k + 1) * q],
                            )
                    else:
                        nc.sync.dma_start(out=feats_sb[:, h, :], in_=feats[c, h])

                for t in range(TILES_PER_CHUNK):
                    tg = c * TILES_PER_CHUNK + t
                    col = t * P

                    # proj[pix, c] = sum_f fT[f, pix] * w_aug[f, c]
                    proj_ps = psum_pool.tile([P, CP], F32, tag="projps")
                    nc.tensor.matmul(
                        out=proj_ps[:],
                        lhsT=feats_sb[:, 0, col : col + P],
                        rhs=w_sb[:, 0, :],
                        start=True,
                        stop=False,
                    )
                    nc.tensor.matmul(
                        out=proj_ps[:],
                        lhsT=feats_sb[:, 1, col : col + P],
                        rhs=w_sb[:, 1, :],
                        start=False,
                        stop=True,
                    )
                    pq_sb = work_pool.tile([P, CP], BF16, tag="pqsb")
                    nc.scalar.activation(
                        out=pq_sb[:],
                        in_=proj_ps[:],
                        func=mybir.ActivationFunctionType.Copy,
                    )

                    # onehot[p, s] = (iota[p, s] == label[p]); all-bf16
                    # bit-pattern compare (exact)
                    onehot = work_pool.tile([P, NUM_SP], BF16, tag="onehot")
                    nc.vector.tensor_scalar(
                        onehot[:],
                        iota_sb[:],
                        labels_sb[:, tg : tg + 1],
                        None,
                        mybir.AluOpType.is_equal,
                    )

                    # acc[32g + c, s'] += pq[pix, c] * onehot[pix, 512h + s']
                    for half in range(2):
                        g = 2 * (tg % 2) + half
                        row = 32 * g
                        nc.tensor.matmul(
                            out=acc[row : row + CP, :],
                            lhsT=pq_sb[:],
                            rhs=onehot[:, 512 * half : 512 * (half + 1)],
                            start=tg < 2,
                            stop=tg >= N_TILES - 2,
                            tile_position=(0, row),
                            skip_group_check=True,
                        )

            out_sb = chunk_pool.tile([P, 512], F32, tag="outsb")
            nc.scalar.activation(
                out=out_sb[:], in_=acc[:], func=mybir.ActivationFunctionType.Copy
            )
            nc.sync.dma_start(out=out[:], in_=out_sb[:])

    nc.compile()
    return nc


def _install_ntff_hook():
    """Register the axon NTFF profiling hook when the image's antenv
    lacks axon_hooks (mirrors trn_agent_boot._ntff_profile_via_ctypes)."""
    import contextlib
    import ctypes
    import sys
    import types

    if "antenv.axon_hooks" in sys.modules:
        return
    lib = ctypes.CDLL("/opt/axon/libaxon_pjrt.so")
    if not hasattr(lib, "axon_start_nrt_profile"):
        return
    lib.axon_start_nrt_profile.argtypes = [
        ctypes.POINTER(ctypes.c_int64),
        ctypes.c_size_t,
    ]
    lib.axon_start_nrt_profile.restype = ctypes.c_int64
    lib.axon_stop_nrt_profile.argtypes = [ctypes.c_char_p]
    lib.axon_stop_nrt_profile.restype = ctypes.c_int64

    @contextlib.contextmanager
    def _hook(output_dir, device_ids):
        import jax

        jax.devices()
        if device_ids:
            ids = (ctypes.c_int64 * len(device_ids))(*device_ids)
            rc = lib.axon_start_nrt_profile(ids, len(device_ids))
        else:
            rc = lib.axon_start_nrt_profile(None, 0)
        if rc != 0:
            raise RuntimeError(f"axon_start_nrt_profile rc={rc}")
        try:
            yield
        finally:
            n = lib.axon_stop_nrt_profile(str(output_dir).encode())
            print(f"profile: {n} file(s) written to {output_dir}", file=sys.stderr)

    mod = types.ModuleType("antenv.axon_hooks")
    mod.get_axon_ntff_profile_hook = lambda: _hook
    mod.set_axon_ntff_profile_hook = lambda h: None
    sys.modules["antenv.axon_hooks"] = mod


_NC_CACHE = None


def _get_nc():
    global _NC_CACHE
    if _NC_CACHE is None:
        _NC_CACHE = _build_nc()
    return _NC_CACHE


def _encode_bf16_pattern(v):
    """Map small non-negative ints to distinct, exactly-comparable bf16
    bit patterns (0x4000 + v are all normal, distinct values)."""
    return (0x4000 + np.asarray(v, dtype=np.uint16)).view(ml_dtypes.bfloat16)


def kernel(features, superpixel, w_node):
    features = np.asarray(features, dtype=np.float32)
    superpixel = np.asarray(superpixel)
    w_node = np.asarray(w_node, dtype=np.float32)

    feats_flat = features.reshape(NPIX, F)
    sp_flat = superpixel.reshape(NPIX).astype(np.int64)

    # host-side layout: transposed bf16 features [256 f, NPIX pix]
    fT = np.ascontiguousarray(feats_flat.astype(ml_dtypes.bfloat16).T)
    enc = _encode_bf16_pattern(sp_flat)

    wa = np.zeros((F, CP), dtype=np.float32)
    wa[:, :C] = w_node.T
    # w_aug dram layout [P, 2, CP]: [f_lo, h, c] = w_aug[128h + f_lo, c]
    wa_bf = np.ascontiguousarray(
        wa.astype(ml_dtypes.bfloat16).reshape(2, P, CP).transpose(1, 0, 2)
    )
    iota = np.ascontiguousarray(
        np.broadcast_to(_encode_bf16_pattern(np.arange(NUM_SP))[None, :], (P, NUM_SP))
    )

    in_maps = []
    for core in range(N_CORES):
        lo = core * PIX_PER_CORE
        fc = fT[:, lo : lo + PIX_PER_CORE]
        # feats[c, h, f, j] = fT[128h + f, lo + c*CHUNK_PIX + j]
        f_dev = np.ascontiguousarray(
            fc.reshape(2, P, N_CHUNKS, CHUNK_PIX).transpose(2, 0, 1, 3)
        )
        # labels[p, tg] = enc(sp[lo + 128*tg + p]); scalar port is fp32,
        # bf16->fp32 is exact so the pattern compare still matches
        lab = np.ascontiguousarray(
            enc[lo : lo + PIX_PER_CORE].reshape(N_TILES, P).T.astype(np.float32)
        )
        in_maps.append(
            {"feats": f_dev, "labels": lab, "iota": iota, "w_aug": wa_bf}
        )

    trace = bool(int(_os.environ.get("KERNEL_TRACE", "0")))
    repeat = int(_os.environ.get("KERNEL_REPEAT", "1"))
    kwargs = {}
    if trace:
        _install_ntff_hook()
        import concourse.bass_utils as _bu

        _bu.upload_artifacts = lambda tmpdir: tmpdir
    base_dir = _os.environ.get("KERNEL_TRACE_DIR") or None
    for rep in range(repeat):
        if trace and base_dir:
            kwargs["tmpdir"] = _os.path.join(base_dir, f"rep{rep}")
            _os.makedirs(kwargs["tmpdir"], exist_ok=True)
        res = run_bass_kernel_spmd(
            _get_nc(), in_maps, core_ids=list(range(N_CORES)), trace=trace, **kwargs
        )
        if trace:
            print(f"HW exec time: {res.exec_time_ns} ns")
            print(f"profile_json: {res.profile_json}")

    total = np.zeros((C, NUM_SP), dtype=np.float64)
    for r in res.results:
        o = np.asarray(r["out"], dtype=np.float64)
        total[:, 0:512] += o[0:C] + o[64 : 64 + C]
        total[:, 512:1024] += o[32 : 32 + C] + o[96 : 96 + C]
    counts = np.bincount(sp_flat, minlength=NUM_SP).astype(np.float64)
    node_potentials = total / np.clip(counts, 1.0, None)
    return np.ascontiguousarray(node_potentials.T).astype(np.float32)
